# revision 1
# baseline (speedup 1.0000x reference)
"""Trainium2 Bass kernel for nn_ConcatBlock (dense_mlp).

Computes, for x:(4,512,256,64) f32 and s:(4,256) f32:
    xt   = x transposed to (b,t,h,c)
    z    = concat([xt, s bcast], -1) @ W.T + b        # (b,t,h,512)
    z    = LayerNorm(PReLU(z, a2), ln2_w, ln2_b)       # over last dim, eps=1e-8
    y    = xt + z ; output = y transposed back to (b,c,t,h)

Sharding: data-parallel over 8 NeuronCores — each core takes one batch and
half the T dimension (8192 tokens), params replicated. Fully self-contained.
"""
import os
import sys
import numpy as np

B, C1, T, H, AUX, OUT = 4, 512, 256, 64, 256, 512
EPS = 1e-8
N_CORES = 8
TOK_PER_CORE = (T // 2) * H          # 8192
ST_TOK = 512                         # tokens per supertile
N_ST = TOK_PER_CORE // ST_TOK        # 16
N_CHUNK = ST_TOK // 128              # 4 chunks of 128 tokens

LAST_EXEC_TIME_NS = None
_CACHE = {}


def _apply_tile_patch():
    """walrus in this container caps CTRL (Drain) instructions at one sync
    wait; Tile's exit barrier attaches every outstanding wait to a single
    Drain. Split them across a chain of single-wait Drains (SP executes
    them sequentially, so the combined effect is identical)."""
    import concourse.tile as tile
    from concourse import mybir
    from concourse.vector_clock import ScopedClock

    if getattr(tile.TileContext, "_drain_split_patched", False):
        return

    def _drain_and_barrier(self, tick_clock, wait_clock):
        drain_inst = self.nc.sync.drain()
        wait_clock.add_sem_waits(
            drain_inst.ins, ScopedClock({None: tick_clock.global_clock})
        )
        si = drain_inst.ins.sync_info
        if si is not None and si.on_wait is not None and len(si.on_wait) > 1:
            waits = list(si.on_wait)
            drain_inst.ins.sync_info = mybir.SyncInfo(
                on_wait=[waits[0]], on_update=list(si.on_update or [])
            )
            for w in waits[1:]:
                d2 = self.nc.sync.drain()
                d2.ins.sync_info = mybir.SyncInfo(on_wait=[w], on_update=[])
        self.nc.all_engine_barrier()
        assert self.sems is not None
        popped = self.nc._tile_sem_poison_stack.pop()
        assert popped is self._sem_poison
        self.nc.clear_and_free_semaphores(list(self.sems.allocated().values()))
        self.nc.all_engine_barrier()

    tile.TileContext._drain_and_barrier = _drain_and_barrier
    tile.TileContext._drain_split_patched = True


def _ensure_ntff_hook():
    """Provide antenv.axon_hooks (absent in this container) so that
    run_bass_kernel_spmd(trace=True) can capture NTFF profiles."""
    import types
    import ctypes
    import contextlib

    if "antenv.axon_hooks" in sys.modules:
        return
    mod = types.ModuleType("antenv.axon_hooks")
    _state = {"hook": None}

    so_path = "/opt/axon/libaxon_pjrt.so"
    try:
        lib = ctypes.CDLL(so_path)
        if hasattr(lib, "axon_start_nrt_profile"):
            lib.axon_start_nrt_profile.argtypes = [
                ctypes.POINTER(ctypes.c_int64),
                ctypes.c_size_t,
            ]
            lib.axon_start_nrt_profile.restype = ctypes.c_int64
            lib.axon_stop_nrt_profile.argtypes = [ctypes.c_char_p]
            lib.axon_stop_nrt_profile.restype = ctypes.c_int64

            @contextlib.contextmanager
            def _hook(output_dir, device_ids):
                import jax

                jax.devices()
                if device_ids:
                    ids = (ctypes.c_int64 * len(device_ids))(*device_ids)
                    rc = lib.axon_start_nrt_profile(ids, len(device_ids))
                else:
                    rc = lib.axon_start_nrt_profile(None, 0)
                if rc != 0:
                    raise RuntimeError(f"axon_start_nrt_profile rc={rc}")
                try:
                    yield
                finally:
                    n = lib.axon_stop_nrt_profile(str(output_dir).encode())
                    if n < 0:
                        raise RuntimeError(f"axon_stop_nrt_profile rc={n}")

            _state["hook"] = _hook
    except OSError:
        pass

    mod.get_axon_ntff_profile_hook = lambda: _state["hook"]
    mod.set_axon_ntff_profile_hook = lambda h: _state.__setitem__("hook", h)
    sys.modules["antenv.axon_hooks"] = mod


def _split_multi_waits(nc):
    """walrus here caps instructions at ONE sync-wait command. Move extra
    waits onto single-wait NoOps inserted just before, on the same engine
    (engine issue is in-order, so blocking earlier is equivalent)."""
    from concourse import mybir

    for fn in nc.m.functions:
        for blk in fn.blocks:
            insts = blk.instructions
            out = []
            changed = False
            for inst in insts:
                si = getattr(inst, "sync_info", None)
                if si is not None and si.on_wait is not None and len(si.on_wait) > 1:
                    waits = list(si.on_wait)
                    for w in waits[:-1]:
                        nop = mybir.InstNoOp(
                            name=nc.get_next_instruction_name(), ins=[], outs=[]
                        )
                        nop.engine = inst.engine
                        nop.sync_info = mybir.SyncInfo(on_wait=[w], on_update=[])
                        nc.register_instruction(nop)
                        out.append(nop)
                    inst.sync_info = mybir.SyncInfo(
                        on_wait=[waits[-1]], on_update=list(si.on_update or [])
                    )
                    changed = True
                out.append(inst)
            if changed:
                blk.instructions = out


def _build_program(alpha, apply_wb):
    import concourse.bass as bass
    import concourse.tile as tile
    from concourse import mybir
    from concourse.masks import make_identity

    f32 = mybir.dt.float32
    bf16 = mybir.dt.bfloat16
    nc = bass.Bass()

    x = nc.declare_dram_parameter("x", [C1, TOK_PER_CORE], f32, isOutput=False)
    wx = nc.declare_dram_parameter("wx", [C1, OUT], bf16, isOutput=False)
    ws = nc.declare_dram_parameter("ws", [AUX, OUT], f32, isOutput=False)
    sb = nc.declare_dram_parameter("sb", [128, 2], f32, isOutput=False)
    bv = nc.declare_dram_parameter("bv", [1, OUT], f32, isOutput=False)
    if apply_wb:
        lnw = nc.declare_dram_parameter("lnw", [1, OUT], f32, isOutput=False)
        lnb = nc.declare_dram_parameter("lnb", [1, OUT], f32, isOutput=False)
    y = nc.declare_dram_parameter("y", [C1, TOK_PER_CORE], f32, isOutput=True)

    xv = x.rearrange("(c p) t -> c p t", p=128)     # [4,128,8192]
    wv = wx.rearrange("(c p) o -> c p o", p=128)    # [4,128,512]
    wsv = ws.rearrange("(c p) o -> c p o", p=128)   # [2,128,512]
    yv = y.rearrange("(j p) t -> j p t", p=128)     # [4,128,8192]

    Prelu = mybir.ActivationFunctionType.Prelu
    Ident = mybir.ActivationFunctionType.Identity
    Sqrt = mybir.ActivationFunctionType.Sqrt

    with tile.TileContext(nc) as tc:
        with (
            tc.tile_pool(name="consts", bufs=1) as consts,
            tc.tile_pool(name="xin", bufs=3) as xin,
            tc.tile_pool(name="work", bufs=3) as work,
            tc.tile_pool(name="yout", bufs=2) as yout,
            tc.tile_pool(name="small", bufs=8) as small,
            tc.tile_pool(name="zps", bufs=3, space="PSUM") as zps,
            tc.tile_pool(name="yps", bufs=1, space="PSUM") as yps,
        ):
            # ---- one-time setup ----
            w_sb = consts.tile([128, 4, OUT], bf16)
            for c in range(4):
                nc.sync.dma_start(out=w_sb[:, c, :], in_=wv[c])
            ws_sb = consts.tile([128, 2, OUT], f32)
            for c in range(2):
                nc.sync.dma_start(out=ws_sb[:, c, :], in_=wsv[c])
            s_sb = consts.tile([128, 2], f32)
            nc.sync.dma_start(out=s_sb[:], in_=sb[:])
            b_sb = consts.tile([1, OUT], f32)
            nc.sync.dma_start(out=b_sb[:], in_=bv[:])
            ones_sb = consts.tile([128, 128], bf16)
            nc.vector.memset(ones_sb, 1.0)
            ident = consts.tile([128, 128], f32)
            make_identity(nc, ident)
            eps_t = consts.tile([128, 1], f32)
            nc.vector.memset(eps_t, EPS)
            if apply_wb:
                import concourse.bass as _b
                lnw_rep = consts.tile([128, OUT], f32)
                nc.sync.dma_start(
                    out=lnw_rep,
                    in_=_b.AP(tensor=lnw.tensor, offset=lnw.offset,
                              ap=[[0, 128], [1, OUT]]),
                )
                lnb_rep = consts.tile([128, OUT], f32)
                nc.sync.dma_start(
                    out=lnb_rep,
                    in_=_b.AP(tensor=lnb.tensor, offset=lnb.offset,
                              ap=[[0, 128], [1, OUT]]),
                )

            # z_s = Ws.T-contracted with s (per this core's batch) + bias,
            # staged as row 0 of zeroed [128, OUT] tiles so the mainline can
            # add it with all-ones matmuls. The GEMM operands are bf16, so
            # the f32 bias row is split exactly into bf16 hi + lo parts.
            zrow = consts.tile([128, OUT], f32)
            nc.vector.memset(zrow, 0.0)
            zrow_b = consts.tile([128, OUT], bf16)
            nc.vector.memset(zrow_b, 0.0)
            with tc.tile_pool(name="setup_ps", bufs=1, space="PSUM") as sps:
                zs_p = sps.tile([1, OUT], f32)
                nc.tensor.matmul(zs_p, lhsT=s_sb[:, 0:1], rhs=ws_sb[:, 0, :],
                                 start=True, stop=False)
                nc.tensor.matmul(zs_p, lhsT=s_sb[:, 1:2], rhs=ws_sb[:, 1, :],
                                 start=False, stop=True)
                nc.vector.tensor_add(out=zrow[0:1, :], in0=zs_p[:], in1=b_sb[:])
            # exact f32 bias row as bf16 hi (row 0) + lo (row 1); the all-ones
            # stationary operand sums both rows in a single matmul.
            nc.vector.tensor_copy(out=zrow_b[0:1, :], in_=zrow[0:1, :])
            zrow_hi_f = consts.tile([1, OUT], f32)
            nc.vector.tensor_copy(out=zrow_hi_f, in_=zrow_b[0:1, :])
            zrow_lo_f = consts.tile([1, OUT], f32)
            nc.vector.tensor_tensor(out=zrow_lo_f, in0=zrow[0:1, :],
                                    in1=zrow_hi_f, op=mybir.AluOpType.subtract)
            zrow_lo_b = consts.tile([1, OUT], bf16)
            nc.vector.tensor_copy(out=zrow_lo_b, in_=zrow_lo_f)
            # compute engines can't target a partition base of 1; DMA can
            nc.sync.dma_start(out=zrow_b[1:2, :], in_=zrow_lo_b)

            # ---- main loop ----
            for st in range(N_ST):
                tok0 = st * ST_TOK
                x_t = xin.tile([128, 4, ST_TOK], f32)
                for c in range(4):
                    nc.sync.dma_start(out=x_t[:, c, :],
                                      in_=xv[c, :, tok0:tok0 + ST_TOK])
                xb = xin.tile([128, 4, ST_TOK], bf16, tag="xb")
                nc.scalar.copy(out=xb, in_=x_t)
                yT = yps.tile([128, N_CHUNK, OUT], f32)
                y_t = yout.tile([128, 4, ST_TOK], f32)
                for i in range(N_CHUNK):
                    z = zps.tile([128, OUT], f32)
                    for c in range(4):
                        nc.tensor.matmul(
                            z, lhsT=xb[:, c, i * 128:(i + 1) * 128],
                            rhs=w_sb[:, c, :], start=(c == 0), stop=False)
                    nc.tensor.matmul(z, lhsT=ones_sb, rhs=zrow_b,
                                     start=False, stop=True)

                    zp = work.tile([128, OUT], f32, tag="zp")
                    nc.scalar.activation(out=zp, in_=z, func=Prelu,
                                         bias=0.0, scale=1.0, alpha=alpha)

                    stats = small.tile([128, 6], f32, tag="stats")
                    nc.vector.bn_stats(out=stats, in_=zp)
                    mv = small.tile([128, 2], f32, tag="mv")
                    nc.vector.bn_aggr(out=mv, in_=stats)
                    std = small.tile([128, 1], f32, tag="std")
                    nc.scalar.activation(out=std, in_=mv[:, 1:2], func=Sqrt,
                                         bias=eps_t)
                    rstd = small.tile([128, 1], f32, tag="rstd")
                    nc.vector.reciprocal(out=rstd, in_=std)
                    numer = small.tile([128, 1], f32, tag="numer")
                    nc.vector.tensor_scalar(
                        out=numer, in0=mv[:, 0:1], scalar1=rstd, scalar2=-1.0,
                        op0=mybir.AluOpType.mult, op1=mybir.AluOpType.mult)

                    zn = work.tile([128, OUT], f32, tag="zn")
                    nc.scalar.activation(out=zn, in_=zp, func=Ident,
                                         bias=numer, scale=rstd)
                    if apply_wb:
                        zn2 = work.tile([128, OUT], f32, tag="zn2")
                        nc.vector.tensor_mul(out=zn2, in0=zn, in1=lnw_rep)
                        nc.vector.tensor_add(out=zn2, in0=zn2, in1=lnb_rep)
                        zn = zn2

                    for j in range(4):
                        nc.tensor.transpose(
                            yT[:, i, j * 128:(j + 1) * 128],
                            zn[:, j * 128:(j + 1) * 128], ident)

                xr = x_t.rearrange("p c (i t) -> p c i t", t=128)
                yr = y_t.rearrange("p c (i t) -> p c i t", t=128)
                for j in range(4):
                    nc.vector.tensor_add(
                        out=yr[:, j], in0=yT[:, :, j * 128:(j + 1) * 128],
                        in1=xr[:, j])
                for j in range(4):
                    nc.sync.dma_start(out=yv[j, :, tok0:tok0 + ST_TOK],
                                      in_=y_t[:, j, :])
    _split_multi_waits(nc)
    return nc


def kernel(**inputs):
    global LAST_EXEC_TIME_NS
    _apply_tile_patch()
    _ensure_ntff_hook()
    from concourse.bass_utils import run_bass_kernel_spmd

    x = np.asarray(inputs["x"], dtype=np.float32)
    s = np.asarray(inputs["s"], dtype=np.float32)
    W = np.asarray(inputs["W"], dtype=np.float32)
    b = np.asarray(inputs["b"], dtype=np.float32)
    alpha = float(np.asarray(inputs["prelu2_a"]))
    ln2_w = np.asarray(inputs["ln2_w"], dtype=np.float32)
    ln2_b = np.asarray(inputs["ln2_b"], dtype=np.float32)
    apply_wb = not (np.all(ln2_w == 1.0) and np.all(ln2_b == 0.0))

    key = (alpha, apply_wb)
    if key not in _CACHE:
        _CACHE[key] = _build_program(alpha, apply_wb)
    nc = _CACHE[key]

    import ml_dtypes

    WT = np.ascontiguousarray(W.T)            # [768, 512]
    wx = np.ascontiguousarray(WT[:C1]).astype(ml_dtypes.bfloat16)  # [512, 512]
    ws = WT[C1:]                              # [256, 512]
    bv = np.ascontiguousarray(b.reshape(1, OUT))

    in_maps = []
    for core in range(N_CORES):
        bi, th = core // 2, core % 2
        xs = np.ascontiguousarray(
            x[bi, :, th * (T // 2):(th + 1) * (T // 2), :]
        ).reshape(C1, TOK_PER_CORE)
        sbm = np.ascontiguousarray(s[bi].reshape(2, 128).T)  # sb[p,j]=s[128j+p]
        m = {"x": xs, "wx": wx, "ws": ws, "sb": sbm, "bv": bv}
        if apply_wb:
            m["lnw"] = np.ascontiguousarray(ln2_w.reshape(1, OUT))
            m["lnb"] = np.ascontiguousarray(ln2_b.reshape(1, OUT))
        in_maps.append(m)

    trace = bool(int(os.environ.get("KERNEL_TRACE", "0")))
    kw = {}
    if trace:
        kw["trace"] = True
        kw["tmpdir"] = os.environ.get("KERNEL_TRACE_DIR") or None
    res = run_bass_kernel_spmd(nc, in_maps, core_ids=list(range(N_CORES)), **kw)
    LAST_EXEC_TIME_NS = res.exec_time_ns

    out = np.empty((B, C1, T, H), dtype=np.float32)
    for core in range(N_CORES):
        bi, th = core // 2, core % 2
        out[bi, :, th * (T // 2):(th + 1) * (T // 2), :] = (
            res.results[core]["y"].reshape(C1, T // 2, H)
        )
    return out

